# revision 1
# baseline (speedup 1.0000x reference)
"""Head-sharded (tensor-parallel) CrossAttention kernel for 8 trn2 NeuronCores.

Problem shapes (hardcoded): B=2, L=2048, QD=1024, H=16, D=64.
Each core owns 2 heads end-to-end (proj -> QK-RMSNorm -> RoPE -> attention
-> output projection partial); the all-reduce over cores happens on host.

All matmuls run in float32r (fp32 storage, TF32-like rounding, 1 cycle/row
on the PE for moving sizes >= 256).  Per-core dataflow:

  Phase P(b): x^T tiles (stationary) x Wqkv^T (moving) -> qkv [bl,384] PSUM.
     Sum-of-squares for QK RMS-norm is fused into the PSUM->SBUF staging
     copy (DVE tensor_tensor_reduce).  rrms via one batched ACT Sqrt + DVE
     reciprocal.  RoPE on DVE with host-precomputed coefficient planes
     (q_scale/k_scale folded in; even/odd pairs live in the free dim so all
     ops are full 128-partition).  q-hat/k-hat transposed on PE into
     [m, bl]; v staged as [bl, v|1] with a ones column so the o^T matmul
     also produces the softmax denominator.
  Phase A(b): scores^T [lk=128, lq=1024] = two N=512 matmuls into a 2-bank
     PSUM tile; one wide exp per tile via ACT, with per-partition scale
     rrms_k/sqrt(D) (k's norm is folded into the exp argument - softmax is
     computed without max-subtraction, safe since |scores| < 20).
     o^T accumulates [v|1]^T x expS over lk (K=128); row 64 = denominator;
     normalize = DVE recip + GPSIMD partition-broadcast + DVE mul.
     Phase A(b) overlaps Phase P(b+1) across engines.
  Phase O: out[bl,e] partial = o^T (stationary) x Wproj^T (moving);
     PSUM evacuated to bf16 (split DVE/ACT) and DMA'd out.  Host sums the
     8 partials in fp32 and adds bproj.
"""

import numpy as np

import concourse.bass as bass
import concourse.tile as tile
from concourse import bacc, mybir
from concourse.bass_utils import run_bass_kernel_spmd
from concourse.masks import make_identity

F32 = mybir.dt.float32
F32R = mybir.dt.float32r
BF16 = mybir.dt.bfloat16
AF = mybir.ActivationFunctionType
ALU = mybir.AluOpType

B, L, QD, H, D = 2, 2048, 1024, 16, 64
INNER = H * D
NCORES = 8
HL = H // NCORES          # heads per core = 2
M = HL * D                # 128 head-dim rows per core
BL = B * L                # 4096
NT = BL // 128            # 32 bl-tiles
NTH = NT // 2             # 16 tiles per batch
CT = QD // 128            # 8 contraction tiles
LQC = 1024                # lq chunk (2 psum banks)
NLQ = L // LQC            # 2
NLK = L // 128            # 16 lk chunks

_CACHE = {}


def _build_nc():
    nc = bacc.Bacc("TRN2", target_bir_lowering=False, debug=False)

    xt = nc.dram_tensor("xt", [NT, 128, CT, 128], F32R, kind="ExternalInput")
    wqkv = nc.dram_tensor("wqkv", [128, CT, 3 * M], F32R, kind="ExternalInput")
    wproj = nc.dram_tensor("wproj", [M, QD], F32R, kind="ExternalInput")
    coefs = nc.dram_tensor("coefs", [NLK, 128, 4 * M], F32, kind="ExternalInput")
    outp = nc.dram_tensor("outp", [BL, QD], BF16, kind="ExternalOutput")

    with tile.TileContext(nc) as tc:
        with (
            tc.tile_pool(name="res", bufs=1) as res,
            tc.tile_pool(name="xs", bufs=3) as xs,
            tc.tile_pool(name="cf", bufs=3) as cf,
            tc.tile_pool(name="stg", bufs=3) as stg,
            tc.tile_pool(name="wk", bufs=3) as wk,
            tc.tile_pool(name="nrm", bufs=2) as nrm,
            tc.tile_pool(name="es", bufs=4) as esp,
            tc.tile_pool(name="ob", bufs=3) as obp,
            tc.tile_pool(name="pA", bufs=2, space="PSUM") as pA,   # big: 2 banks/buf
            tc.tile_pool(name="pB", bufs=1, space="PSUM") as pB,   # o: 2 banks
        ):
            # ---- residents ----
            wqkv_sb = res.tile([128, CT, 3 * M], F32R)
            nc.sync.dma_start(out=wqkv_sb, in_=wqkv[:, :, :])
            wproj_sb = res.tile([M, QD], F32R)
            nc.sync.dma_start(out=wproj_sb, in_=wproj[:, :])

            qhT = res.tile([M, BL], F32R)
            khT = res.tile([M, BL], F32R)
            vaug = res.tile([128, NT, 2 * (D + 1)], F32R)
            oT = res.tile([M, BL], F32R)
            rr = res.tile([128, NT, 4], F32)

            ones_f = res.tile([128, NT], F32)
            nc.vector.memset(ones_f, 1.0)
            nc.vector.tensor_copy(
                vaug[:, :, D : D + 1].rearrange("p a b -> p (a b)"), ones_f
            )
            nc.vector.tensor_copy(
                vaug[:, :, 2 * D + 1 : 2 * D + 2].rearrange("p a b -> p (a b)"),
                ones_f,
            )
            ident_f = res.tile([128, 128], F32)
            make_identity(nc, ident_f)
            identr = res.tile([128, 128], F32R)
            nc.vector.tensor_copy(identr, ident_f)
            magic = res.tile([128, 16], mybir.dt.int32)
            nc.vector.memset(magic, 0x5F3759DF)

            NSB = 4  # tiles per P sub-batch; 4 sub-batches per b

            def emit_P_subbatch(bb, sb):
                qk_st = stg.tile([128, NSB, 2 * M], F32, tag="stage")
                ssq = stg.tile([128, NSB, 4], F32, tag="ssq")
                for t in range(NSB):
                    jj = sb * NSB + t
                    j = bb * NTH + jj
                    xt_t = xs.tile([128, CT, 128], F32R)
                    nc.sync.dma_start(out=xt_t, in_=xt[j, :, :, :])
                    ps = pA.tile([128, 3 * M], F32, tag="proj")
                    for ci in range(CT):
                        nc.tensor.matmul(
                            ps,
                            lhsT=xt_t[:, ci, :],
                            rhs=wqkv_sb[:, ci, :],
                            start=(ci == 0),
                            stop=(ci == CT - 1),
                        )
                    nc.vector.tensor_copy(qk_st[:, t, :], ps[:, 0 : 2 * M])
                    sqs = wk.tile([128, 2 * M], F32, tag="sqscratch")
                    nc.gpsimd.tensor_mul(sqs, qk_st[:, t, :], qk_st[:, t, :])
                    nc.vector.reduce_sum(
                        out=ssq[:, t, :].rearrange("p (a b) -> p a b", b=1),
                        in_=sqs.rearrange("p (a b) -> p a b", a=4),
                        axis=mybir.AxisListType.X,
                    )
                    nc.scalar.copy(vaug[:, j, 0:D], ps[:, 2 * M : 2 * M + D])
                    nc.scalar.copy(
                        vaug[:, j, D + 1 : 2 * D + 1], ps[:, 2 * M + D : 3 * M]
                    )

                # rrms via Newton rsqrt on DVE (rr cols 2:4 hold rrms_k/8)
                j0 = bb * NTH + sb * NSB
                rrs = rr[:, j0 : j0 + NSB, :]
                nx = wk.tile([128, NSB, 4], F32, tag="nx")
                nc.vector.tensor_scalar(
                    out=nx[:, :, 0:2], in0=ssq[:, :, 0:2],
                    scalar1=1.0 / D, scalar2=1e-6, op0=ALU.mult, op1=ALU.add,
                )
                nc.vector.tensor_scalar(
                    out=nx[:, :, 2:4], in0=ssq[:, :, 2:4],
                    scalar1=1.0, scalar2=float(D) * 1e-6, op0=ALU.mult, op1=ALU.add,
                )
                sh = wk.tile([128, NSB, 4], mybir.dt.int32, tag="nsh")
                nc.vector.tensor_scalar(
                    out=sh, in0=nx.bitcast(mybir.dt.int32), scalar1=1,
                    scalar2=None, op0=ALU.logical_shift_right,
                )
                nc.vector.tensor_tensor(
                    out=rrs.bitcast(mybir.dt.int32),
                    in0=magic[:, 0 : NSB * 4].rearrange("p (a b) -> p a b", b=4),
                    in1=sh,
                    op=ALU.subtract,
                )
                ht = wk.tile([128, NSB, 4], F32, tag="nht")
                for _ in range(2):  # y *= 1.5 - 0.5*x*y*y
                    nc.vector.tensor_mul(ht, nx, rrs)
                    nc.vector.tensor_mul(ht, ht, rrs)
                    nc.vector.tensor_scalar(
                        out=ht, in0=ht, scalar1=-0.5, scalar2=1.5,
                        op0=ALU.mult, op1=ALU.add,
                    )
                    nc.vector.tensor_mul(rrs, rrs, ht)

                for t in range(NSB):
                    jj = sb * NSB + t
                    j = bb * NTH + jj
                    for g in range(2):  # normalize q in place
                        nc.gpsimd.tensor_scalar_mul(
                            qk_st[:, t, g * D : (g + 1) * D],
                            qk_st[:, t, g * D : (g + 1) * D],
                            rr[:, j, g : g + 1],
                        )
                    cft = cf.tile([128, 4 * M], F32)
                    nc.sync.dma_start(out=cft, in_=coefs[jj, :, :])

                    qk = wk.tile([128, 2 * M], F32R, tag="ropeout")
                    t1 = wk.tile([128, M], F32, tag="ropetmp")
                    src = qk_st[:, t, :].rearrange(
                        "p (g d2 two) -> p g d2 two", g=4, two=2
                    )
                    dst = qk.rearrange("p (g d2 two) -> p g d2 two", g=4, two=2)
                    pl = [
                        cft[:, i * M : (i + 1) * M].rearrange(
                            "p (g d2) -> p g d2", g=4
                        )
                        for i in range(4)
                    ]
                    t1v = t1.rearrange("p (g d2) -> p g d2", g=4)
                    ev, od = src[:, :, :, 0], src[:, :, :, 1]
                    nc.vector.tensor_mul(dst[:, :, :, 0], ev, pl[0])
                    nc.vector.tensor_mul(t1v, od, pl[1])
                    nc.vector.tensor_add(dst[:, :, :, 0], dst[:, :, :, 0], t1v)
                    nc.vector.tensor_mul(dst[:, :, :, 1], ev, pl[2])
                    nc.vector.tensor_mul(t1v, od, pl[3])
                    nc.vector.tensor_add(dst[:, :, :, 1], dst[:, :, :, 1], t1v)

                    for which, dest in ((0, qhT), (1, khT)):
                        pst = pA.tile([128, 128], F32R, tag="proj")
                        nc.tensor.transpose(
                            pst, qk[:, which * M : (which + 1) * M], identr
                        )
                        if which == 0:
                            nc.vector.tensor_copy(
                                dest[:, j * 128 : (j + 1) * 128], pst
                            )
                        else:
                            nc.scalar.copy(dest[:, j * 128 : (j + 1) * 128], pst)

            def emit_A_combo(bb, lq, h):
                qs = qhT[
                    h * D : (h + 1) * D,
                    bb * L + lq * LQC : bb * L + (lq + 1) * LQC,
                ]
                po = pB.tile([D + 1, LQC], F32, tag="o")
                for lk in range(NLK):
                    j = bb * NLK + lk
                    pss = pA.tile([128, LQC], F32, tag="big")
                    for half in range(2):
                        nc.tensor.matmul(
                            pss[:, half * 512 : (half + 1) * 512],
                            lhsT=khT[
                                h * D : (h + 1) * D,
                                bb * L + lk * 128 : bb * L + (lk + 1) * 128,
                            ],
                            rhs=qs[:, half * 512 : (half + 1) * 512],
                            start=True,
                            stop=True,
                        )
                    es = esp.tile([128, LQC], F32R, tag="es")
                    nc.scalar.activation(
                        out=es, in_=pss, func=AF.Exp,
                        scale=rr[:, j, 2 + h : 3 + h],
                    )
                    for half in range(2):
                        nc.tensor.matmul(
                            po[:, half * 512 : (half + 1) * 512],
                            lhsT=vaug[:, j, h * (D + 1) : (h + 1) * (D + 1)],
                            rhs=es[:, half * 512 : (half + 1) * 512],
                            start=(lk == 0),
                            stop=(lk == NLK - 1),
                            skip_group_check=True,
                        )
                rd = nrm.tile([1, LQC], F32, tag="rd")
                nc.vector.reciprocal(rd, po[D : D + 1, :])
                rdb = nrm.tile([D, LQC], F32, tag="rdb")
                nc.gpsimd.partition_broadcast(rdb, rd)
                nc.vector.tensor_mul(
                    oT[
                        h * D : (h + 1) * D,
                        bb * L + lq * LQC : bb * L + (lq + 1) * LQC,
                    ],
                    po[0:D, :],
                    rdb,
                )

            def emit_O_chunk(bb, lq):
                ntpc = LQC // 128  # 8 bl-tiles per lq chunk
                for j in range(bb * NTH + lq * ntpc, bb * NTH + (lq + 1) * ntpc):
                    ps = pA.tile([128, QD], F32, tag="big")
                    for eo in range(2):
                        nc.tensor.matmul(
                            ps[:, eo * 512 : (eo + 1) * 512],
                            lhsT=oT[:, j * 128 : (j + 1) * 128],
                            rhs=wproj_sb[:, eo * 512 : (eo + 1) * 512],
                            start=True,
                            stop=True,
                        )
                    ob = obp.tile([128, QD], BF16, tag="ob")
                    nc.vector.tensor_copy(ob[:, 0:512], ps[:, 0:512])
                    nc.scalar.copy(ob[:, 512:1024], ps[:, 512:1024])
                    nc.sync.dma_start(out=outp[j * 128 : (j + 1) * 128, :], in_=ob)

            for bb in range(B):
                for sb in range(4):
                    emit_P_subbatch(bb, sb)
                for h in range(HL):
                    for lq in range(NLQ):
                        emit_A_combo(bb, lq, h)
                for lq in range(NLQ):
                    emit_O_chunk(bb, lq)

    nc.compile()
    return nc


def _prep_inputs(x, pe, Wq, Wkv, Wproj, q_scale, k_scale):
    x = np.asarray(x, np.float32)
    xT = np.ascontiguousarray(x.reshape(BL, QD).T)                    # [QD, BL]
    xtt = np.ascontiguousarray(
        xT.reshape(CT, 128, NT, 128).transpose(2, 1, 0, 3)
    )                                                                 # [NT, p, CT, n]

    pe = np.asarray(pe, np.float32)[0, 0]                             # [L, 32, 2, 2]
    qs, ks = np.asarray(q_scale, np.float32), np.asarray(k_scale, np.float32)

    def planes(scale):
        se, so = scale[0::2], scale[1::2]
        return (
            pe[:, :, 0, 0] * se[None, :],
            pe[:, :, 0, 1] * so[None, :],
            pe[:, :, 1, 0] * se[None, :],
            pe[:, :, 1, 1] * so[None, :],
        )

    pq, pk = planes(qs), planes(ks)
    coefs = np.empty((L, 4, 4, 32), np.float32)                       # [l, plane, grp, d2]
    for p_i in range(4):
        coefs[:, p_i, 0] = pq[p_i]
        coefs[:, p_i, 1] = pq[p_i]
        coefs[:, p_i, 2] = pk[p_i]
        coefs[:, p_i, 3] = pk[p_i]
    coefs = np.ascontiguousarray(coefs.reshape(NLK, 128, 4 * M))

    Wq = np.asarray(Wq, np.float32)
    Wkv = np.asarray(Wkv, np.float32)
    Wproj = np.asarray(Wproj, np.float32)
    Wk_full, Wv_full = Wkv[:INNER], Wkv[INNER:]

    in_maps = []
    for c in range(NCORES):
        r0, r1 = c * M, (c + 1) * M
        wqkv_c = np.concatenate([Wq[r0:r1], Wk_full[r0:r1], Wv_full[r0:r1]], axis=0)
        wqkv_t = np.ascontiguousarray(
            wqkv_c.T.reshape(CT, 128, 3 * M).transpose(1, 0, 2)
        )                                                             # [128, CT, 3M]
        wproj_c = np.ascontiguousarray(Wproj[:, r0:r1].T)             # [M, QD]
        in_maps.append(
            {"xt": xtt, "wqkv": wqkv_t, "wproj": wproj_c, "coefs": coefs}
        )
    return in_maps


def kernel(x, pe, Wq, Wkv, Wproj, bproj, q_scale, k_scale):
    if "nc" not in _CACHE:
        _CACHE["nc"] = _build_nc()
    nc = _CACHE["nc"]
    in_maps = _prep_inputs(x, pe, Wq, Wkv, Wproj, q_scale, k_scale)
    res = run_bass_kernel_spmd(nc, in_maps, core_ids=list(range(NCORES)))
    acc = np.zeros((BL, QD), np.float32)
    for c in range(NCORES):
        acc += res.results[c]["outp"].astype(np.float32)
    acc += np.asarray(bproj, np.float32)[None, :]
    return acc.reshape(B, L, QD)

